# revision 4
# baseline (speedup 1.0000x reference)
"""Trainium2 Bass kernel for nn_CombineModel_wo_net (histogram_binning).

Full inputs in, full output out. Internally: data-parallel across 8
NeuronCores, 2 images per core. Each core streams its 2x3x544x960 fp32
slice from HBM (12.53 MB -> ~35.0 us at the 358 GB/s HBM-per-NC limit)
and reduces it to per-partition partials:
  - sum of s = c0+c1+c2 per pixel          (for avg brightness)
  - bright/dark threshold counts against s in {2.25, 0.75}
The tiny [5,16] epilogue (dynamic-range ratio, gap select, exposure
where-chains) is replicated exactly in float32 numpy on the host from
the gathered partials.

Engine split (DMA is the roofline; keep both compute engines well under
it so the stream never stalls):
  - DVE: t = c0+c1 (tensor_tensor, 1x mode) and s = t+c2 with fused
    row-sum accumulate (scalar_tensor_tensor) -> ~2.08 ns/col, vs DMA
    4.29 ns/col.
  - ACT: the two threshold counts as Sign(s - T) with fused row-sum
    accumulate -> ~1.67 ns/col. Host decodes cnt_ge = (N + sum_sign)/2;
    a pixel with s == T contributes 0, leaving the partial at K - 0.5,
    fixed by round-half-up (verified: comparing s vs 3T is exact w.r.t.
    the reference's mean(c) >= T; see threshold note below).
  - The last DVE_TAIL chunks run their counts on DVE (tensor_scalar
    is_ge, 2x mode) instead: after the final DMA byte lands, only
    ~1 us of tapered DVE work remains, while ACT's ~0.3 us/op fixed
    cost would stretch the drain.

The chunk plan ramps up at the start (fast pipeline fill: first TT can
start ~1.2 us in) and tapers at the end (per-chunk compute fits under
the remaining DMA stream), keeping the HW time near the HBM roofline.

Threshold equivalence note: comparing s = c0+c1+c2 against 3*T is exact
w.r.t. the reference's g = mean(c) >= T because fp32 spacing at s~3T is
wider than the rounding interval of s/3 (or s*(1/3)) around T for
T in {0.25, 0.75}; no representable s straddles the thresholds.
"""

import sys

for _p in ("/opt/trn_rl_repo",):
    if _p not in sys.path:
        sys.path.insert(0, _p)

from contextlib import ExitStack

import numpy as np

import concourse.bass as bass
import concourse.bacc as bacc
import concourse.mybir as mybir
import concourse.tile as tile
from concourse.bass_utils import run_bass_kernel_spmd

# Problem geometry (hardcoded per contract).
B, C, H, W = 16, 3, 544, 960
N_CORES = 8
IMGS_PER_CORE = B // N_CORES          # 2
PLANE = H * W                          # 522240 = 128 * 4080
P = 128
COLS = PLANE // P                      # 4080
NQ = 3                                 # sum_s, cnt_ge_2.25, cnt_ge_0.75

# Per-image column splits: ramp up (fill), taper down (drain).
PLAN = [[204, 408, 1020, 1224, 1224], [1836, 1020, 612, 408, 204]]
DVE_TAIL = 3                           # last N chunks count on DVE, not ACT

F32 = mybir.dt.float32
BF16 = mybir.dt.bfloat16

# Module-level knobs (test.py pokes these; grading path uses defaults).
TRACE = False
LAST_RESULT = None  # BassKernelResults of most recent run (for profiling)

_compiled_nc = None


def _build_bass(reps=1, body_copies=1, in_bufs=5, s_bufs=4, dve_tail=DVE_TAIL,
                plan=None):
    """Emit the per-core Tile program (same SPMD program on all 8 cores).

    reps > 1 wraps the workload in a hardware For_i loop so one NEFF
    execution runs it `reps * body_copies` times; the bench harness uses
    marginal time per iteration as the HW exec time. The grading path
    uses reps=1, body_copies=1 (no loop).
    """
    if plan is None:
        plan = PLAN
    chunks = [sz for p in plan for sz in p]
    n_chunks = len(chunks)
    nacc = n_chunks * NQ
    nc = bacc.Bacc(
        "TRN2", target_bir_lowering=False, debug=False, num_devices=N_CORES
    )
    img = nc.dram_tensor(
        "img", [IMGS_PER_CORE, C, P, COLS], F32, kind="ExternalInput"
    ).ap()
    # Two accumulator surfaces: acc written only by DVE ops, acc2 only by
    # ACT ops -- avoids any cross-engine same-tile hazard serialization.
    acc_out = nc.dram_tensor("acc", [P, nacc], F32, kind="ExternalOutput").ap()
    acc2_out = nc.dram_tensor("acc2", [P, nacc], F32, kind="ExternalOutput").ap()

    add = mybir.AluOpType.add
    is_ge = mybir.AluOpType.is_ge
    sign = mybir.ActivationFunctionType.Sign

    with ExitStack() as ctx:
        tc = ctx.enter_context(tile.TileContext(nc))
        pool_in = ctx.enter_context(tc.tile_pool(name="inp", bufs=in_bufs))
        pool_t = ctx.enter_context(tc.tile_pool(name="tpool", bufs=2))
        pool_s = ctx.enter_context(tc.tile_pool(name="spool", bufs=s_bufs))
        pool_j = ctx.enter_context(tc.tile_pool(name="junk", bufs=2))
        pool_acc = ctx.enter_context(tc.tile_pool(name="accsb", bufs=1))

        acc_sb = pool_acc.tile([P, nacc], F32, tag="acc")
        acc2_sb = pool_acc.tile([P, nacc], F32, tag="acc2")
        # Per-partition bias vectors for the ACT Sign ops (the activation
        # datapath computes func(in*scale + bias) with bias as a const AP).
        bias_b = pool_acc.tile([P, 1], F32, tag="bias_b")
        bias_q = pool_acc.tile([P, 1], F32, tag="bias_q")
        nc.vector.memset(bias_b[:], -2.25)
        nc.vector.memset(bias_q[:], -0.75)

        def workload():
            g = 0
            for i in range(IMGS_PER_CORE):
                start = 0
                for sz in plan[i]:
                    sl = slice(start, start + sz)
                    start += sz
                    c0 = pool_in.tile([P, sz], F32, tag="c0")
                    nc.sync.dma_start(c0[:], img[i, 0, :, sl])
                    c1 = pool_in.tile([P, sz], F32, tag="c1")
                    nc.sync.dma_start(c1[:], img[i, 1, :, sl])
                    c2 = pool_in.tile([P, sz], F32, tag="c2")
                    nc.sync.dma_start(c2[:], img[i, 2, :, sl])

                    t = pool_t.tile([P, sz], F32, tag="t")
                    nc.vector.tensor_tensor(t[:], c0[:], c1[:], add)
                    # s = (t + 0.0) + c2, fused row-sum into acc column
                    s = pool_s.tile([P, sz], F32, tag="s")
                    nc.vector.scalar_tensor_tensor(
                        s[:], t[:], 0.0, c2[:], add, add,
                        accum_out=acc_sb[:, 3 * g : 3 * g + 1],
                    )
                    if g >= n_chunks - dve_tail:
                        b1 = pool_j.tile([P, sz], F32, tag="jv")
                        nc.vector.tensor_scalar(
                            b1[:], s[:], 2.25, None, is_ge, add,
                            accum_out=acc_sb[:, 3 * g + 1 : 3 * g + 2],
                        )
                        b2 = pool_j.tile([P, sz], F32, tag="jv")
                        nc.vector.tensor_scalar(
                            b2[:], s[:], 0.75, None, is_ge, add,
                            accum_out=acc_sb[:, 3 * g + 2 : 3 * g + 3],
                        )
                    else:
                        a1 = pool_j.tile([P, sz], BF16, tag="ja")
                        nc.scalar.activation(
                            a1[:], s[:], sign, bias=bias_b[:],
                            accum_out=acc2_sb[:, 3 * g + 1 : 3 * g + 2],
                        )
                        a2 = pool_j.tile([P, sz], BF16, tag="ja")
                        nc.scalar.activation(
                            a2[:], s[:], sign, bias=bias_q[:],
                            accum_out=acc2_sb[:, 3 * g + 2 : 3 * g + 3],
                        )
                    g += 1

        if reps == 1:
            for _ in range(body_copies):
                workload()
        else:
            with tc.For_i(0, reps, 1):
                for _ in range(body_copies):
                    workload()

        nc.sync.dma_start(acc_out[:, :], acc_sb[:])
        nc.sync.dma_start(acc2_out[:, :], acc2_sb[:])

    nc.compile()
    return nc, nacc


def _get_nc():
    global _compiled_nc
    if _compiled_nc is None:
        _compiled_nc = _build_bass()[0]
    return _compiled_nc


def kernel(batch_images, base_exposure_1, base_exposure_2):
    global LAST_RESULT
    batch_images = np.ascontiguousarray(np.asarray(batch_images, dtype=np.float32))
    be1 = np.asarray(base_exposure_1, dtype=np.float32)
    be2 = np.asarray(base_exposure_2, dtype=np.float32)
    assert batch_images.shape == (B, C, H, W)

    nc = _get_nc()
    shards = batch_images.reshape(N_CORES, IMGS_PER_CORE, C, P, COLS)
    in_maps = [{"img": shards[c]} for c in range(N_CORES)]
    res = run_bass_kernel_spmd(nc, in_maps, list(range(N_CORES)), trace=TRACE)
    LAST_RESULT = res

    # ---- gather/unshard: fold per-partition partials to per-image stats ----
    chunks = [sz for p in PLAN for sz in p]
    n_chunks = len(chunks)
    chunks_per_img = [len(p) for p in PLAN]

    sum_s = np.empty(B, dtype=np.float64)
    cnt_bright = np.empty(B, dtype=np.float64)
    cnt_ge_quarter = np.empty(B, dtype=np.float64)
    for c in range(N_CORES):
        acc = np.asarray(res.results[c]["acc"], dtype=np.float64)
        acc2 = np.asarray(res.results[c]["acc2"], dtype=np.float64)
        g = 0
        for i in range(IMGS_PER_CORE):
            b = c * IMGS_PER_CORE + i
            ss = 0.0
            cb = 0.0
            cq = 0.0
            for _ in range(chunks_per_img[i]):
                sz = chunks[g]
                ss += acc[:, 3 * g].sum()
                if g >= n_chunks - DVE_TAIL:
                    cb += acc[:, 3 * g + 1].sum()
                    cq += acc[:, 3 * g + 2].sum()
                else:
                    n = np.float64(P * sz)
                    cb += (n + acc2[:, 3 * g + 1].sum()) / 2.0
                    cq += (n + acc2[:, 3 * g + 2].sum()) / 2.0
                g += 1
            sum_s[b] = ss
            # Round half-up: a pixel with s == T exactly contributes 0 to the
            # sign sum, leaving the count at K - 0.5; reference counts it.
            cnt_bright[b] = np.floor(cb + 0.5 + 1e-6)
            cnt_ge_quarter[b] = np.floor(cq + 0.5 + 1e-6)

    # ---- epilogue: replicate reference numerics in fp32 ----
    f32 = np.float32
    bright = cnt_bright.astype(np.float32)                     # exact counts
    dark = (np.float64(PLANE) - cnt_ge_quarter).astype(np.float32)
    dr = bright / (dark + f32(1e-5))
    bright_avg = (sum_s / 3.0 / PLANE).astype(np.float32)

    g = f32(0.5)
    conds = [
        (dr > f32(1.0)) & (bright_avg > f32(0.4)) & (bright_avg < f32(0.6)),
        bright_avg <= f32(0.3),
        bright_avg >= f32(0.7),
        (dr <= f32(1.0)) & (bright_avg > f32(0.3)) & (bright_avg < f32(0.7)),
    ]
    vals = [g * f32(2.0), g * f32(0.5), g * f32(0.5), g * f32(0.75)]
    gaps = np.select(conds, vals, f32(0.0)).astype(np.float32)

    bl = bright_avg[-1]
    gl = gaps[-1]
    s_ = f32(1.7)
    e1 = np.where(
        bl <= f32(0.25), be1 + f32(0.5) * gl * s_,
        np.where(bl >= f32(0.75), be1 - f32(0.5) * gl * s_, be1 - f32(0.3) * gl),
    ).astype(np.float32)
    e2 = np.where(
        bl <= f32(0.25), be2 + f32(0.5) * gl * s_,
        np.where(bl >= f32(0.75), be2 - f32(0.5) * gl * s_, be2 + f32(0.7) * gl),
    ).astype(np.float32)

    return np.stack([dr, bright_avg, gaps, e1, e2]).astype(np.float32)
